# revision 13
# baseline (speedup 1.0000x reference)
"""Multi-head attention + output projection + residual + LayerNorm for Trainium2.

Self-contained SPMD bass kernel over 8 NeuronCores.

Problem (hardcoded): bs=2, N=2048, L=1024, H=16, DK=DV=64, eps=1e-5.
  Q = q@Wq+bq ; K = k@Wk+bk ; V = v@Wv+bv       (per batch)
  attn = softmax(Q K^T / 8)                      -> output #2 [bs,H,N,N] fp32
  A = attn @ V  (bs,H,N,DV)
  A2 = A.reshape(bs, N, H*DV)   # faithful torch bug: no transpose before view!
  Y = A2@Wa + ba + q ; out = LN(Y)*gamma+beta    -> output #1

The buggy reshape maps A2[n, 64*m+d] = A[h=n//128, 16*(n%128)+m, d], i.e.
output row n depends on ONE head (n//128) but ALL its queries. Hence:

Sharding (no collectives): core c = (b=c//4, g=c%4) handles batch b and heads
4g..4g+3 over ALL 2048 queries. That gives the core exactly attn[b, 4g:4g+4]
(67MB slice of the big output) and exactly output rows 512g..512g+512.

Per (head, 512-query chunk) on-chip dataflow:
  scoresT[k,q] = KT_h^T QT_h (fp32r matmul) -> ACT exp(s/8) -> probsT fp16
  AV: psum[65,512] += V_av[128k, 65(fp16, ones col)]^T probsT  (16 k-tiles)
     -> A^T[64,512] unnormalized + per-query sums in row 64
  r = 1/sums ; A^T *= outer(ones64, r) -> AT fp16 ; lnneg = Ln(r^T) (PE transp)
  scores[q,k] = QT_h^T KT_h -> ACT exp(s/8 + lnneg) = normalized probs fp32
     -> DMA to attn output (dominant 67MB/core write)
Out-proj per head: Y[128, L] = sum_m AT_h[:, m::16]^T Wa[64m:64m+64, :] (fp16),
then +ba +q residual, LayerNorm via bn_stats/bn_aggr.
"""

import numpy as np

BS, N, L = 2, 2048, 1024
H, DK, DV = 16, 64, 64
HG = 4  # heads per core
NQ = 512  # output rows per core
EPS = 1e-5
P = 128

_CACHED_NC = None


def _build_nc():
    import concourse.bacc as bacc
    import concourse.bass as bass
    import concourse.mybir as mybir
    import concourse.tile as tile
    from concourse.masks import make_identity

    f32 = mybir.dt.float32
    f32r = mybir.dt.float32r
    f16 = mybir.dt.float16
    AF = mybir.ActivationFunctionType
    OP = mybir.AluOpType

    FS = HG * DK  # 256 sliced projection width

    nc = bacc.Bacc(None, target_bir_lowering=False, debug=False)
    with tile.TileContext(nc) as tc:
        # ---------------- DRAM I/O (per-core slices fed from host) --------
        q_d = nc.dram_tensor("q_full", [N, L], f32, kind="ExternalInput")
        qres_d = nc.dram_tensor("q_res", [NQ, L], f32, kind="ExternalInput")
        k_d = nc.dram_tensor("k_full", [N, L], f32, kind="ExternalInput")
        v_d = nc.dram_tensor("v_full", [N, L], f32, kind="ExternalInput")
        wq_d = nc.dram_tensor("Wq_s", [L, FS], f32, kind="ExternalInput")
        wk_d = nc.dram_tensor("Wk_s", [L, FS], f32, kind="ExternalInput")
        wv_d = nc.dram_tensor("Wv_s", [L, FS], f32, kind="ExternalInput")
        wa_d = nc.dram_tensor("Wa", [L, L], f32, kind="ExternalInput")
        bq_d = nc.dram_tensor("bq_s", [FS], f32, kind="ExternalInput")
        bk_d = nc.dram_tensor("bk_s", [FS], f32, kind="ExternalInput")
        bv_d = nc.dram_tensor("bv_s", [FS], f32, kind="ExternalInput")
        ba_d = nc.dram_tensor("ba", [L], f32, kind="ExternalInput")
        gamma_d = nc.dram_tensor("gamma", [L], f32, kind="ExternalInput")
        beta_d = nc.dram_tensor("beta", [L], f32, kind="ExternalInput")
        attn_d = nc.dram_tensor("attn", [HG, N, N], f32, kind="ExternalOutput")
        y_d = nc.dram_tensor("y", [NQ, L], f32, kind="ExternalOutput")

        def bcast_row(dram_t, n):
            # [n] dram vector -> [128, n] broadcast AP (0-stride partitions)
            return bass.AP(tensor=dram_t, offset=0, ap=[[0, P], [1, n]])

        with (
            tc.tile_pool(name="consts", bufs=1) as consts,
            tc.tile_pool(name="big", bufs=1) as big,
            # PSUM: gen 2 + scT 2x2banks + A 1 + rt 1 = 8 banks
            tc.tile_pool(name="ps_gen", bufs=1, space="PSUM") as ps_gen,
            tc.tile_pool(name="ps_scT", bufs=2, space="PSUM") as ps_scT,
            tc.tile_pool(name="ps_A", bufs=2, space="PSUM") as ps_A,
            tc.tile_pool(name="ps_rt", bufs=1, space="PSUM") as ps_rt,
        ):
            # ---------------- constants ----------------
            ident = consts.tile([P, P], f32)
            make_identity(nc, ident)
            eps_t = consts.tile([P, 1], f32)
            nc.vector.memset(eps_t, EPS)
            ones64 = consts.tile([1, 64], f32)
            nc.vector.memset(ones64, 1.0)
            bq_t = consts.tile([P, 2], f32)
            nc.sync.dma_start(bq_t, bq_d.ap().rearrange("(ft p) -> p ft", p=P))
            bk_t = consts.tile([P, 2], f32)
            nc.sync.dma_start(bk_t, bk_d.ap().rearrange("(ft p) -> p ft", p=P))
            bv_bc = consts.tile([P, FS], f32)
            nc.gpsimd.dma_start(bv_bc, bcast_row(bv_d, FS))

            # persistent through attention
            KT = big.tile([P, 2, N], f32r)         # K^T [f, tok] 16KB/part
            QT = big.tile([P, 2, N], f32r)         # Q^T [f, tok] 16KB/part
            V_av = big.tile([P, 16, HG, 65], f16)  # [k, ktile, head, d+1] 8.3KB
            # A^T normalized, stored permuted: AT[d, h, m, a] = A^T_h[d, 16a+m]
            AT = big.tile([64, HG, 16, P], f16)    # 16KB/part (base-0 only)
            wa_h = big.tile([64, 16, L], f16)      # Wa[64m+d,l] at part d; 32KB
            nc.vector.memset(V_av[:, :, :, 64:65], 1.0)

            # ---------------- helper: transpose [128,512] block ------------
            def transpose_into(dst, src_tile, col_block):
                # dst: [128, 4, 128] slice (f32r); src [128,1024] natural fp32
                pt = ps_gen.tile([P, 4, P], f32, tag="gen")
                for j in range(4):
                    lo = col_block * 4 + j
                    nc.tensor.transpose(
                        pt[:, j, :], src_tile[:, lo * P : (lo + 1) * P], ident
                    )
                nc.vector.tensor_copy(dst, pt)

            # ================= projection phase (scoped pools) =============
            with (
                tc.tile_pool(name="wstage", bufs=2) as wstage,
                tc.tile_pool(name="wr", bufs=1) as wr,
                tc.tile_pool(name="xnat", bufs=3) as xnat,
                tc.tile_pool(name="xtc", bufs=2) as xtc,
            ):
                def load_weight_rounded(w_dram, dst, width):
                    for lo in range(8):
                        st = wstage.tile([P, width], f32, tag="wst")
                        nc.sync.dma_start(
                            st,
                            w_dram.ap().rearrange("(lo p) f -> p lo f", p=P)[:, lo, :],
                        )
                        nc.vector.tensor_copy(dst[:, lo, :], st)

                wq_r = wr.tile([P, 8, FS], f32r, tag="wq")
                wk_r = wr.tile([P, 8, FS], f32r, tag="wk")
                wv_r = wr.tile([P, 8, FS], f32r, tag="wv")
                load_weight_rounded(wk_d, wk_r, FS)
                load_weight_rounded(wv_d, wv_r, FS)
                load_weight_rounded(wq_d, wq_r, FS)
                for m in range(16):
                    st = wstage.tile([64, L], f32, tag="wstm")
                    nc.sync.dma_start(st, wa_d.ap()[m * 64 : (m + 1) * 64, :])
                    nc.vector.tensor_copy(wa_h[:, m, :], st)

                # ---- K^T and Q^T projections (chunked over 512 tokens) ----
                for src_d, w_r, b_t, dst in (
                    (k_d, wk_r, bk_t, KT),
                    (q_d, wq_r, bq_t, QT),
                ):
                    for tch in range(4):
                        xc = xtc.tile([P, 8, 512], f32r, tag="xtc")
                        for i in range(4):
                            tt = tch * 4 + i
                            xn = xnat.tile([P, L], f32, tag="xnat")
                            nc.sync.dma_start(
                                xn, src_d.ap()[tt * P : (tt + 1) * P, :]
                            )
                            for half in range(2):
                                transpose_into(
                                    xc[:, half * 4 : (half + 1) * 4, i * P : (i + 1) * P],
                                    xn,
                                    half,
                                )
                        for ft in range(2):
                            pp = ps_gen.tile([P, 512], f32, tag="gen")
                            for lo in range(8):
                                nc.tensor.matmul(
                                    pp,
                                    w_r[:, lo, ft * P : (ft + 1) * P],
                                    xc[:, lo, :],
                                    start=(lo == 0),
                                    stop=(lo == 7),
                                )
                            nc.vector.tensor_scalar_add(
                                dst[:, ft, tch * 512 : (tch + 1) * 512],
                                pp,
                                b_t[:, ft : ft + 1],
                            )

                # ---- V projection (natural layout + ones col) ----
                for kt_i in range(16):
                    vnat = xnat.tile([P, L], f32, tag="xnat")
                    nc.sync.dma_start(vnat, v_d.ap()[kt_i * P : (kt_i + 1) * P, :])
                    vtc = xtc.tile([P, 8, P], f32r, tag="xtc")
                    for half in range(2):
                        transpose_into(vtc[:, half * 4 : (half + 1) * 4, :], vnat, half)
                    pp = ps_gen.tile([P, 512], f32, tag="gen")
                    for lo in range(8):
                        nc.tensor.matmul(
                            pp[:, 0:FS],
                            vtc[:, lo, :],
                            wv_r[:, lo, :],
                            start=(lo == 0),
                            stop=(lo == 7),
                        )
                    nc.vector.tensor_tensor(
                        V_av[:, kt_i, :, 0:64],
                        pp[:, 0:FS].rearrange("p (h d) -> p h d", d=64),
                        bv_bc.rearrange("p (h d) -> p h d", d=64),
                        OP.add,
                    )

            # ---------------- attention + per-head out-proj/LN -------------
            scale = 1.0 / float(np.sqrt(DK))
            with (
                tc.tile_pool(name="pt16", bufs=4) as pt16,
                tc.tile_pool(name="small", bufs=3) as small,
                tc.tile_pool(name="stage", bufs=3) as stage_pool,
                tc.tile_pool(name="lnc", bufs=1) as lnc,
                tc.tile_pool(name="lnp", bufs=2) as lnp,
                tc.tile_pool(name="lns", bufs=2) as lns,
            ):
                ba_bc = lnc.tile([P, L], f32)
                nc.gpsimd.dma_start(ba_bc, bcast_row(ba_d, L))
                g_bc = lnc.tile([P, L], f32)
                nc.gpsimd.dma_start(g_bc, bcast_row(gamma_d, L))
                b_bc = lnc.tile([P, L], f32)
                nc.gpsimd.dma_start(b_bc, bcast_row(beta_d, L))

                for h in range(HG):
                    ht2, hp2 = h // 2, (h % 2) * 64
                    # ---- phase A: scoresT -> probsT -> AV (all 4 chunks) --
                    at_scrs, r_rows = [], []
                    rT_all = small.tile([P, 4, 4], f32, tag="rT", bufs=2)
                    for qc in range(4):  # 512-query chunks
                        QT_hc = QT[hp2 : hp2 + 64, ht2, qc * 512 : (qc + 1) * 512]
                        psA = ps_A.tile([65, 512], f32, tag="psA")
                        for kk in range(8):  # pairs of key tiles
                            ps2 = ps_scT.tile([P, 2, 512], f32, tag="scT")
                            for j in range(2):
                                kt_i = kk * 2 + j
                                nc.tensor.matmul(
                                    ps2[:, j, :],
                                    KT[hp2 : hp2 + 64, ht2, kt_i * P : (kt_i + 1) * P],
                                    QT_hc,
                                    start=True,
                                    stop=True,
                                )
                            ptile = pt16.tile([P, 2, 512], f16, tag="pt")
                            nc.scalar.activation(ptile, ps2, AF.Exp, scale=scale)
                            for j in range(2):
                                kt_i = kk * 2 + j
                                nc.tensor.matmul(
                                    psA,
                                    V_av[:, kt_i, h, :],
                                    ptile[:, j, :],
                                    start=(kt_i == 0),
                                    stop=(kt_i == 15),
                                )
                        sums_row = small.tile([1, 512], f32, tag="sums", bufs=6)
                        nc.vector.tensor_copy(sums_row, psA[64:65, :])
                        at_scr = small.tile([64, 512], f32, tag="atscr", bufs=6)
                        nc.vector.tensor_copy(at_scr, psA[0:64, :])
                        at_scrs.append(at_scr)
                        # sums^T [128,4] -> reciprocal (full-width, fast)
                        psT = ps_rt.tile([P, 4], f32, tag="rt")
                        for qt_j in range(4):
                            nc.tensor.transpose(
                                psT[:, qt_j : qt_j + 1],
                                sums_row[:, qt_j * P : (qt_j + 1) * P],
                                ident[0:1, 0:1],
                            )
                        rT = rT_all[:, qc, :]
                        nc.vector.reciprocal(rT, psT)
                        # transpose back -> r_row [1,512] for the R-outer
                        psB = ps_rt.tile([1, 4, P], f32, tag="rt")
                        for qt_j in range(4):
                            nc.tensor.transpose(
                                psB[:, qt_j, :], rT[:, qt_j : qt_j + 1], ident
                            )
                        r_row = small.tile([1, 512], f32, tag="rrow", bufs=6)
                        nc.vector.tensor_copy(r_row, psB.rearrange("p a b -> p (a b)"))
                        r_rows.append(r_row)

                    # ---- mid: one batched Ln per head; R-outer + AT evict --
                    lnneg_all = small.tile([P, 4, 4], f32, tag="lnneg", bufs=2)
                    nc.scalar.activation(lnneg_all, rT_all, AF.Ln)
                    for qc in range(4):
                        psR = ps_rt.tile([64, 512], f32, tag="rt")
                        nc.tensor.matmul(
                            psR, ones64, r_rows[qc], start=True, stop=True
                        )
                        nc.vector.tensor_tensor(
                            AT[:, h, :, qc * 32 : (qc + 1) * 32],
                            at_scrs[qc].rearrange("p (a m) -> p m a", m=16),
                            psR.rearrange("p (a m) -> p m a", m=16),
                            OP.mult,
                        )

                    # ---- phase B: scores -> normalized probs -> DMA -------
                    for qc in range(4):
                        QT_hc = QT[hp2 : hp2 + 64, ht2, qc * 512 : (qc + 1) * 512]
                        for qt_j in range(4):
                            qt_g = qc * 4 + qt_j
                            stg = stage_pool.tile([P, N], f32, tag="probs")
                            for kc in range(2):
                                ps3 = ps_scT.tile([P, 2, 512], f32, tag="scT")
                                for j in range(2):
                                    kc_i = kc * 2 + j
                                    nc.tensor.matmul(
                                        ps3[:, j, :],
                                        QT_hc[:, qt_j * P : (qt_j + 1) * P],
                                        KT[
                                            hp2 : hp2 + 64,
                                            ht2,
                                            kc_i * 512 : (kc_i + 1) * 512,
                                        ],
                                        start=True,
                                        stop=True,
                                    )
                                nc.scalar.activation(
                                    stg[:, kc * 1024 : (kc + 1) * 1024],
                                    ps3,
                                    AF.Exp,
                                    scale=scale,
                                    bias=lnneg_all[:, qc, qt_j : qt_j + 1],
                                )
                            nc.sync.dma_start(
                                attn_d.ap()[h, qt_g * P : (qt_g + 1) * P, :], stg
                            )

                for h in range(HG):
                    # ---- out-proj for head h (buggy-reshape routing) ----
                    # Y[a, l] = sum_{m,d} AT_h[d, 16a+m] * Wa[64m+d, l]
                    AT_h = AT[:, h, :, :]
                    y_t = lnp.tile([P, L], f32, tag="y")
                    for lc in range(2):
                        pp = ps_gen.tile([P, 512], f32, tag="gen")
                        for m in range(16):
                            nc.tensor.matmul(
                                pp,
                                AT_h[:, m, :],
                                wa_h[:, m, lc * 512 : (lc + 1) * 512],
                                start=(m == 0),
                                stop=(m == 15),
                            )
                        nc.vector.tensor_tensor(
                            y_t[:, lc * 512 : (lc + 1) * 512],
                            pp,
                            ba_bc[:, lc * 512 : (lc + 1) * 512],
                            OP.add,
                        )
                    qres = lnp.tile([P, L], f32, tag="qres")
                    nc.sync.dma_start(qres, qres_d.ap()[h * P : (h + 1) * P, :])
                    nc.vector.tensor_add(y_t, y_t, qres)
                    stats = lns.tile([P, 2, 6], f32, tag="stats")
                    for sg in range(2):
                        nc.vector.bn_stats(
                            stats[:, sg, :], y_t[:, sg * 512 : (sg + 1) * 512]
                        )
                    mv = lns.tile([P, 2], f32, tag="mv")
                    nc.vector.bn_aggr(mv, stats)
                    std = lns.tile([P, 1], f32, tag="std")
                    nc.scalar.activation(std, mv[:, 1:2], AF.Sqrt, bias=eps_t)
                    rstd = lns.tile([P, 1], f32, tag="rstd")
                    nc.vector.reciprocal(rstd, std)
                    nc.vector.tensor_scalar(
                        y_t, y_t, mv[:, 0:1], rstd, OP.subtract, OP.mult
                    )
                    nc.vector.tensor_tensor(y_t, y_t, g_bc, OP.mult)
                    nc.vector.tensor_tensor(y_t, y_t, b_bc, OP.add)
                    nc.sync.dma_start(y_d.ap()[h * P : (h + 1) * P, :], y_t)

    nc.compile()
    return nc


def _get_nc():
    global _CACHED_NC
    if _CACHED_NC is None:
        _CACHED_NC = _build_nc()
    return _CACHED_NC


def kernel(q, k, v, Wq, bq, Wk, bk, Wv, bv, Wa, ba, gamma, beta):
    from concourse.bass_utils import run_bass_kernel_spmd

    def cc(a):
        return np.ascontiguousarray(np.asarray(a, dtype=np.float32))

    q, k, v = cc(q), cc(k), cc(v)
    Wq, Wk, Wv, Wa = cc(Wq), cc(Wk), cc(Wv), cc(Wa)
    bq, bk, bv, ba = cc(bq), cc(bk), cc(bv), cc(ba)
    gamma, beta = cc(gamma), cc(beta)

    FS = HG * DK
    in_maps = []
    for c in range(8):
        b, g = c // 4, c % 4
        fs = slice(g * FS, (g + 1) * FS)
        in_maps.append(
            {
                "q_full": q[b],
                "q_res": np.ascontiguousarray(q[b, g * NQ : (g + 1) * NQ, :]),
                "k_full": k[b],
                "v_full": v[b],
                "Wq_s": np.ascontiguousarray(Wq[:, fs]),
                "Wk_s": np.ascontiguousarray(Wk[:, fs]),
                "Wv_s": np.ascontiguousarray(Wv[:, fs]),
                "Wa": Wa,
                "bq_s": np.ascontiguousarray(bq[fs]),
                "bk_s": np.ascontiguousarray(bk[fs]),
                "bv_s": np.ascontiguousarray(bv[fs]),
                "ba": ba,
                "gamma": gamma,
                "beta": beta,
            }
        )

    nc = _get_nc()
    results = run_bass_kernel_spmd(nc, in_maps, core_ids=list(range(8))).results

    attention = np.empty((BS, H, N, N), np.float32)
    output = np.empty((BS, N, L), np.float32)
    for c in range(8):
        b, g = c // 4, c % 4
        attention[b, g * HG : (g + 1) * HG, :, :] = results[c]["attn"]
        output[b, g * NQ : (g + 1) * NQ, :] = results[c]["y"]
    return output, attention


# revision 14
# speedup vs baseline: 1.5127x; 1.5127x over previous
"""Multi-head attention + output projection + residual + LayerNorm for Trainium2.

Self-contained SPMD bass kernel over 8 NeuronCores.

Problem (hardcoded): bs=2, N=2048, L=1024, H=16, DK=DV=64, eps=1e-5.
  Q = q@Wq+bq ; K = k@Wk+bk ; V = v@Wv+bv       (per batch)
  attn = softmax(Q K^T / 8)                      -> output #2 [bs,H,N,N] fp32
  A = attn @ V  (bs,H,N,DV)
  A2 = A.reshape(bs, N, H*DV)   # faithful torch bug: no transpose before view!
  Y = A2@Wa + ba + q ; out = LN(Y)*gamma+beta    -> output #1

The buggy reshape maps A2[n, 64*m+d] = A[h=n//128, 16*(n%128)+m, d], i.e.
output row n depends on ONE head (n//128) but ALL its queries. Hence:

Sharding (no collectives): core c = (b=c//4, g=c%4) handles batch b and heads
4g..4g+3 over ALL 2048 queries. That gives the core exactly attn[b, 4g:4g+4]
(67MB slice of the big output) and exactly output rows 512g..512g+512.

Per (head, 512-query chunk) on-chip dataflow:
  scoresT[k,q] = KT_h^T QT_h (fp32r matmul) -> ACT exp(s/8) -> probsT fp16
  AV: psum[65,512] += V_av[128k, 65(fp16, ones col)]^T probsT  (16 k-tiles)
     -> A^T[64,512] unnormalized + per-query sums in row 64
  r = 1/sums ; A^T *= outer(ones64, r) -> AT fp16 ; lnneg = Ln(r^T) (PE transp)
  scores[q,k] = QT_h^T KT_h -> ACT exp(s/8 + lnneg) = normalized probs fp32
     -> DMA to attn output (dominant 67MB/core write)
Out-proj per head: Y[128, L] = sum_m AT_h[:, m::16]^T Wa[64m:64m+64, :] (fp16),
then +ba +q residual, LayerNorm via bn_stats/bn_aggr.
"""

import numpy as np

BS, N, L = 2, 2048, 1024
H, DK, DV = 16, 64, 64
HG = 4  # heads per core
NQ = 512  # output rows per core
EPS = 1e-5
P = 128

_CACHED_NC = None


def _build_nc():
    import concourse.bacc as bacc
    import concourse.bass as bass
    import concourse.mybir as mybir
    import concourse.tile as tile
    from concourse.masks import make_identity

    f32 = mybir.dt.float32
    f32r = mybir.dt.float32r
    f16 = mybir.dt.float16
    AF = mybir.ActivationFunctionType
    OP = mybir.AluOpType

    FS = HG * DK  # 256 sliced projection width

    nc = bacc.Bacc(None, target_bir_lowering=False, debug=False)
    with tile.TileContext(nc) as tc:
        # ---------------- DRAM I/O (per-core slices fed from host) --------
        q_d = nc.dram_tensor("q_full", [N, L], f32, kind="ExternalInput")
        qres_d = nc.dram_tensor("q_res", [NQ, L], f32, kind="ExternalInput")
        k_d = nc.dram_tensor("k_full", [N, L], f32, kind="ExternalInput")
        v_d = nc.dram_tensor("v_full", [N, L], f32, kind="ExternalInput")
        wq_d = nc.dram_tensor("Wq_s", [L, FS], f32, kind="ExternalInput")
        wk_d = nc.dram_tensor("Wk_s", [L, FS], f32, kind="ExternalInput")
        wv_d = nc.dram_tensor("Wv_s", [L, FS], f32, kind="ExternalInput")
        wa_d = nc.dram_tensor("Wa", [L, L], f32, kind="ExternalInput")
        bq_d = nc.dram_tensor("bq_s", [FS], f32, kind="ExternalInput")
        bk_d = nc.dram_tensor("bk_s", [FS], f32, kind="ExternalInput")
        bv_d = nc.dram_tensor("bv_s", [FS], f32, kind="ExternalInput")
        ba_d = nc.dram_tensor("ba", [L], f32, kind="ExternalInput")
        gamma_d = nc.dram_tensor("gamma", [L], f32, kind="ExternalInput")
        beta_d = nc.dram_tensor("beta", [L], f32, kind="ExternalInput")
        attn_d = nc.dram_tensor("attn", [HG, N, N], f32, kind="ExternalOutput")
        y_d = nc.dram_tensor("y", [NQ, L], f32, kind="ExternalOutput")

        def bcast_row(dram_t, n):
            # [n] dram vector -> [128, n] broadcast AP (0-stride partitions)
            return bass.AP(tensor=dram_t, offset=0, ap=[[0, P], [1, n]])

        with (
            tc.tile_pool(name="consts", bufs=1) as consts,
            tc.tile_pool(name="big", bufs=1) as big,
            # PSUM: gen 2 + scT 2x2banks + A 1 + rt 1 = 8 banks
            tc.tile_pool(name="ps_gen", bufs=2, space="PSUM") as ps_gen,
            tc.tile_pool(name="ps_scT", bufs=2, space="PSUM") as ps_scT,
            tc.tile_pool(name="ps_A", bufs=1, space="PSUM") as ps_A,
            tc.tile_pool(name="ps_rt", bufs=1, space="PSUM") as ps_rt,
        ):
            # ---------------- constants ----------------
            ident = consts.tile([P, P], f32)
            make_identity(nc, ident)
            eps_t = consts.tile([P, 1], f32)
            nc.vector.memset(eps_t, EPS)
            ones64 = consts.tile([1, 64], f32)
            nc.vector.memset(ones64, 1.0)
            bq_t = consts.tile([P, 2], f32)
            nc.sync.dma_start(bq_t, bq_d.ap().rearrange("(ft p) -> p ft", p=P))
            bk_t = consts.tile([P, 2], f32)
            nc.sync.dma_start(bk_t, bk_d.ap().rearrange("(ft p) -> p ft", p=P))
            bv_bc = consts.tile([P, FS], f32)
            nc.gpsimd.dma_start(bv_bc, bcast_row(bv_d, FS))

            # persistent through attention
            KT = big.tile([P, 2, N], f32r)         # K^T [f, tok] 16KB/part
            QT = big.tile([P, 2, N], f32r)         # Q^T [f, tok] 16KB/part
            V_av = big.tile([P, 16, HG, 65], f16)  # [k, ktile, head, d+1] 8.3KB
            # A^T normalized, stored permuted: AT[d, h, m, a] = A^T_h[d, 16a+m]
            AT = big.tile([64, HG, 16, P], f16)    # 16KB/part (base-0 only)
            wa_h = big.tile([64, 16, L], f16)      # Wa[64m+d,l] at part d; 32KB
            nc.vector.memset(V_av[:, :, :, 64:65], 1.0)

            # ---------------- helper: transpose [128,512] block ------------
            def transpose_into(dst, src_tile, col_block):
                # dst: [128, 4, 128] slice (f32r); src [128,1024] natural fp32
                pt = ps_gen.tile([P, 4, P], f32, tag="gen")
                for j in range(4):
                    lo = col_block * 4 + j
                    nc.tensor.transpose(
                        pt[:, j, :], src_tile[:, lo * P : (lo + 1) * P], ident
                    )
                nc.vector.tensor_copy(dst, pt)

            # ================= projection phase (scoped pools) =============
            with (
                tc.tile_pool(name="wstage", bufs=2) as wstage,
                tc.tile_pool(name="wr", bufs=1) as wr,
                tc.tile_pool(name="xnat", bufs=3) as xnat,
                tc.tile_pool(name="xtc", bufs=2) as xtc,
            ):
                def load_weight_rounded(w_dram, dst, width):
                    for lo in range(8):
                        st = wstage.tile([P, width], f32, tag="wst")
                        nc.sync.dma_start(
                            st,
                            w_dram.ap().rearrange("(lo p) f -> p lo f", p=P)[:, lo, :],
                        )
                        nc.vector.tensor_copy(dst[:, lo, :], st)

                wq_r = wr.tile([P, 8, FS], f32r, tag="wq")
                wk_r = wr.tile([P, 8, FS], f32r, tag="wk")
                wv_r = wr.tile([P, 8, FS], f32r, tag="wv")
                load_weight_rounded(wk_d, wk_r, FS)
                load_weight_rounded(wv_d, wv_r, FS)
                load_weight_rounded(wq_d, wq_r, FS)
                for m in range(16):
                    st = wstage.tile([64, L], f32, tag="wstm")
                    nc.sync.dma_start(st, wa_d.ap()[m * 64 : (m + 1) * 64, :])
                    nc.vector.tensor_copy(wa_h[:, m, :], st)

                # ---- K^T and Q^T projections (chunked over 512 tokens) ----
                for src_d, w_r, b_t, dst in (
                    (k_d, wk_r, bk_t, KT),
                    (q_d, wq_r, bq_t, QT),
                ):
                    for tch in range(4):
                        xc = xtc.tile([P, 8, 512], f32r, tag="xtc")
                        for i in range(4):
                            tt = tch * 4 + i
                            xn = xnat.tile([P, L], f32, tag="xnat")
                            nc.sync.dma_start(
                                xn, src_d.ap()[tt * P : (tt + 1) * P, :]
                            )
                            for half in range(2):
                                transpose_into(
                                    xc[:, half * 4 : (half + 1) * 4, i * P : (i + 1) * P],
                                    xn,
                                    half,
                                )
                        for ft in range(2):
                            pp = ps_gen.tile([P, 512], f32, tag="gen")
                            for lo in range(8):
                                nc.tensor.matmul(
                                    pp,
                                    w_r[:, lo, ft * P : (ft + 1) * P],
                                    xc[:, lo, :],
                                    start=(lo == 0),
                                    stop=(lo == 7),
                                )
                            nc.vector.tensor_scalar_add(
                                dst[:, ft, tch * 512 : (tch + 1) * 512],
                                pp,
                                b_t[:, ft : ft + 1],
                            )

                # ---- V projection (natural layout + ones col) ----
                for kt_i in range(16):
                    vnat = xnat.tile([P, L], f32, tag="xnat")
                    nc.sync.dma_start(vnat, v_d.ap()[kt_i * P : (kt_i + 1) * P, :])
                    vtc = xtc.tile([P, 8, P], f32r, tag="xtc")
                    for half in range(2):
                        transpose_into(vtc[:, half * 4 : (half + 1) * 4, :], vnat, half)
                    pp = ps_gen.tile([P, 512], f32, tag="gen")
                    for lo in range(8):
                        nc.tensor.matmul(
                            pp[:, 0:FS],
                            vtc[:, lo, :],
                            wv_r[:, lo, :],
                            start=(lo == 0),
                            stop=(lo == 7),
                        )
                    nc.vector.tensor_tensor(
                        V_av[:, kt_i, :, 0:64],
                        pp[:, 0:FS].rearrange("p (h d) -> p h d", d=64),
                        bv_bc.rearrange("p (h d) -> p h d", d=64),
                        OP.add,
                    )

            # ---------------- attention + per-head out-proj/LN -------------
            scale = 1.0 / float(np.sqrt(DK))
            with (
                tc.tile_pool(name="pt16", bufs=4) as pt16,
                tc.tile_pool(name="small", bufs=3) as small,
                tc.tile_pool(name="stage", bufs=3) as stage_pool,
                tc.tile_pool(name="lnc", bufs=1) as lnc,
                tc.tile_pool(name="lnp", bufs=2) as lnp,
                tc.tile_pool(name="lns", bufs=2) as lns,
            ):
                ba_bc = lnc.tile([P, L], f32)
                nc.gpsimd.dma_start(ba_bc, bcast_row(ba_d, L))
                g_bc = lnc.tile([P, L], f32)
                nc.gpsimd.dma_start(g_bc, bcast_row(gamma_d, L))
                b_bc = lnc.tile([P, L], f32)
                nc.gpsimd.dma_start(b_bc, bcast_row(beta_d, L))

                for h in range(HG):
                    ht2, hp2 = h // 2, (h % 2) * 64
                    # ---- phase A: scoresT -> probsT -> AV (all 4 chunks) --
                    at_scrs, r_rows = [], []
                    rT_all = small.tile([P, 4, 4], f32, tag="rT", bufs=2)
                    for qc in range(4):  # 512-query chunks
                        QT_hc = QT[hp2 : hp2 + 64, ht2, qc * 512 : (qc + 1) * 512]
                        psA = ps_A.tile([65, 512], f32, tag="psA")
                        for kk in range(8):  # pairs of key tiles
                            ps2 = ps_scT.tile([P, 2, 512], f32, tag="scT")
                            for j in range(2):
                                kt_i = kk * 2 + j
                                nc.tensor.matmul(
                                    ps2[:, j, :],
                                    KT[hp2 : hp2 + 64, ht2, kt_i * P : (kt_i + 1) * P],
                                    QT_hc,
                                    start=True,
                                    stop=True,
                                )
                            ptile = pt16.tile([P, 2, 512], f16, tag="pt")
                            nc.scalar.activation(ptile, ps2, AF.Exp, scale=scale)
                            for j in range(2):
                                kt_i = kk * 2 + j
                                nc.tensor.matmul(
                                    psA,
                                    V_av[:, kt_i, h, :],
                                    ptile[:, j, :],
                                    start=(kt_i == 0),
                                    stop=(kt_i == 15),
                                )
                        sums_row = small.tile([1, 512], f32, tag="sums", bufs=6)
                        nc.vector.tensor_copy(sums_row, psA[64:65, :])
                        at_scr = small.tile([64, 512], f32, tag="atscr", bufs=6)
                        nc.vector.tensor_copy(at_scr, psA[0:64, :])
                        at_scrs.append(at_scr)
                        # sums^T [128,4] -> reciprocal (full-width, fast)
                        psT = ps_rt.tile([P, 4], f32, tag="rt")
                        for qt_j in range(4):
                            nc.tensor.transpose(
                                psT[:, qt_j : qt_j + 1],
                                sums_row[:, qt_j * P : (qt_j + 1) * P],
                                ident[0:1, 0:1],
                            )
                        rT = rT_all[:, qc, :]
                        nc.vector.reciprocal(rT, psT)
                        # transpose back -> r_row [1,512] for the R-outer
                        psB = ps_rt.tile([1, 4, P], f32, tag="rt")
                        for qt_j in range(4):
                            nc.tensor.transpose(
                                psB[:, qt_j, :], rT[:, qt_j : qt_j + 1], ident
                            )
                        r_row = small.tile([1, 512], f32, tag="rrow", bufs=6)
                        nc.vector.tensor_copy(r_row, psB.rearrange("p a b -> p (a b)"))
                        r_rows.append(r_row)

                    # ---- mid: one batched Ln per head; R-outer + AT evict --
                    lnneg_all = small.tile([P, 4, 4], f32, tag="lnneg", bufs=2)
                    nc.scalar.activation(lnneg_all, rT_all, AF.Ln)
                    for qc in range(4):
                        psR = ps_rt.tile([64, 512], f32, tag="rt")
                        nc.tensor.matmul(
                            psR, ones64, r_rows[qc], start=True, stop=True
                        )
                        nc.vector.tensor_tensor(
                            AT[:, h, :, qc * 32 : (qc + 1) * 32],
                            at_scrs[qc].rearrange("p (a m) -> p m a", m=16),
                            psR.rearrange("p (a m) -> p m a", m=16),
                            OP.mult,
                        )

                    # ---- phase B: scores -> normalized probs -> DMA -------
                    for qc in range(4):
                        QT_hc = QT[hp2 : hp2 + 64, ht2, qc * 512 : (qc + 1) * 512]
                        for qt_j in range(4):
                            qt_g = qc * 4 + qt_j
                            stg = stage_pool.tile([P, N], f32, tag="probs")
                            for kc in range(2):
                                ps3 = ps_scT.tile([P, 2, 512], f32, tag="scT")
                                for j in range(2):
                                    kc_i = kc * 2 + j
                                    nc.tensor.matmul(
                                        ps3[:, j, :],
                                        QT_hc[:, qt_j * P : (qt_j + 1) * P],
                                        KT[
                                            hp2 : hp2 + 64,
                                            ht2,
                                            kc_i * 512 : (kc_i + 1) * 512,
                                        ],
                                        start=True,
                                        stop=True,
                                    )
                                nc.scalar.activation(
                                    stg[:, kc * 1024 : (kc + 1) * 1024],
                                    ps3,
                                    AF.Exp,
                                    scale=scale,
                                    bias=lnneg_all[:, qc, qt_j : qt_j + 1],
                                )
                            nc.sync.dma_start(
                                attn_d.ap()[h, qt_g * P : (qt_g + 1) * P, :], stg
                            )

                for h in range(HG):
                    # ---- out-proj for head h (buggy-reshape routing) ----
                    # Y[a, l] = sum_{m,d} AT_h[d, 16a+m] * Wa[64m+d, l]
                    AT_h = AT[:, h, :, :]
                    y_t = lnp.tile([P, L], f32, tag="y")
                    for lc in range(2):
                        pp = ps_gen.tile([P, 512], f32, tag="gen")
                        for m in range(16):
                            nc.tensor.matmul(
                                pp,
                                AT_h[:, m, :],
                                wa_h[:, m, lc * 512 : (lc + 1) * 512],
                                start=(m == 0),
                                stop=(m == 15),
                            )
                        nc.vector.tensor_tensor(
                            y_t[:, lc * 512 : (lc + 1) * 512],
                            pp,
                            ba_bc[:, lc * 512 : (lc + 1) * 512],
                            OP.add,
                        )
                    qres = lnp.tile([P, L], f32, tag="qres")
                    nc.sync.dma_start(qres, qres_d.ap()[h * P : (h + 1) * P, :])
                    nc.vector.tensor_add(y_t, y_t, qres)
                    stats = lns.tile([P, 2, 6], f32, tag="stats")
                    for sg in range(2):
                        nc.vector.bn_stats(
                            stats[:, sg, :], y_t[:, sg * 512 : (sg + 1) * 512]
                        )
                    mv = lns.tile([P, 2], f32, tag="mv")
                    nc.vector.bn_aggr(mv, stats)
                    std = lns.tile([P, 1], f32, tag="std")
                    nc.scalar.activation(std, mv[:, 1:2], AF.Sqrt, bias=eps_t)
                    rstd = lns.tile([P, 1], f32, tag="rstd")
                    nc.vector.reciprocal(rstd, std)
                    nc.vector.tensor_scalar(
                        y_t, y_t, mv[:, 0:1], rstd, OP.subtract, OP.mult
                    )
                    nc.vector.tensor_tensor(y_t, y_t, g_bc, OP.mult)
                    nc.vector.tensor_tensor(y_t, y_t, b_bc, OP.add)
                    nc.sync.dma_start(y_d.ap()[h * P : (h + 1) * P, :], y_t)

    nc.compile()
    return nc


def _get_nc():
    global _CACHED_NC
    if _CACHED_NC is None:
        _CACHED_NC = _build_nc()
    return _CACHED_NC


def kernel(q, k, v, Wq, bq, Wk, bk, Wv, bv, Wa, ba, gamma, beta):
    from concourse.bass_utils import run_bass_kernel_spmd

    def cc(a):
        return np.ascontiguousarray(np.asarray(a, dtype=np.float32))

    q, k, v = cc(q), cc(k), cc(v)
    Wq, Wk, Wv, Wa = cc(Wq), cc(Wk), cc(Wv), cc(Wa)
    bq, bk, bv, ba = cc(bq), cc(bk), cc(bv), cc(ba)
    gamma, beta = cc(gamma), cc(beta)

    FS = HG * DK
    in_maps = []
    for c in range(8):
        b, g = c // 4, c % 4
        fs = slice(g * FS, (g + 1) * FS)
        in_maps.append(
            {
                "q_full": q[b],
                "q_res": np.ascontiguousarray(q[b, g * NQ : (g + 1) * NQ, :]),
                "k_full": k[b],
                "v_full": v[b],
                "Wq_s": np.ascontiguousarray(Wq[:, fs]),
                "Wk_s": np.ascontiguousarray(Wk[:, fs]),
                "Wv_s": np.ascontiguousarray(Wv[:, fs]),
                "Wa": Wa,
                "bq_s": np.ascontiguousarray(bq[fs]),
                "bk_s": np.ascontiguousarray(bk[fs]),
                "bv_s": np.ascontiguousarray(bv[fs]),
                "ba": ba,
                "gamma": gamma,
                "beta": beta,
            }
        )

    nc = _get_nc()
    results = run_bass_kernel_spmd(nc, in_maps, core_ids=list(range(8))).results

    attention = np.empty((BS, H, N, N), np.float32)
    output = np.empty((BS, N, L), np.float32)
    for c in range(8):
        b, g = c // 4, c % 4
        attention[b, g * HG : (g + 1) * HG, :, :] = results[c]["attn"]
        output[b, g * NQ : (g + 1) * NQ, :] = results[c]["y"]
    return output, attention
